# revision 1
# baseline (speedup 1.0000x reference)
"""Trainium2 Bass kernel for nn_CrossModalAttention (B=4, Sq=Sk=2048, D=512, H=8).

Self-contained: builds an 8-core SPMD Bass/Tile program (one NeuronCore per
(batch, query-half) shard), compiles once per process, runs via
run_bass_kernel_spmd. See build() docstring for the on-chip algorithm.
"""
import sys
sys.path.insert(0, "/opt/trn_rl_repo")
import numpy as np

"""Builder for the CrossModalAttention Trainium2 kernel (one NeuronCore's shard).

Sharding: core c handles batch b=c//2, query-half qh=c%2 (SQ=1024 of 2048 queries).
Cores are fully independent (K/V projection duplicated across the core-pair of a
batch); no collectives.

Layouts (per core; D=512, H=8, DK=64, P=128, DC=D/128=4):
  Qt [128, DC, SQ]  transposed Q-proj; head h lives in chunk h//2, rows (h%2)*64..+64
  Kt [128, DC, SK]  transposed K-proj
  V  [128, KC, H, DK+1] bf16, natural V-proj; column DK is ones (row-sum trick)
  per head h:
    St[k,q]  = Kt_h(kc)^T @ Qt_h          PSUM [128, SQ]  (fp32r matmuls)
    Pt       = exp(St/8)                  ACT -> SBUF bf16
    C_ps     += Pt_qslice^T @ [V_h | 1]   PSUM accum over kc; col DK = rowsum L
    C[:, :, h*64:+64] = C_ps * (1/L)      DVE normalized dump, natural [q, d]
  Ct = PE-transpose(C);  out = Ct^T @ wo + bo + resid -> row LayerNorm -> DRAM

Host-prepped inputs: qT [D,SQ], kT [D,SK], vT [D,SK], resid [SQ,D], wq/wk/wv/wo [D,D],
bq_l/bk_l [128, DC] (per-partition bias layouts), bv_r/bo_r/g_r/b_r [128, D]
(partition-replicated rows). Output: out [SQ, D] f32.
"""

from contextlib import ExitStack

import concourse.bass as bass
import concourse.mybir as mybir
import concourse.tile as tile
from concourse import bacc
from concourse.masks import make_identity

FP32 = mybir.dt.float32
FP32R = mybir.dt.float32r
BF16 = mybir.dt.bfloat16
P = 128
Alu = None


def build(SQ=1024, SK=2048, D=512, H=8, num_devices=8):
    DK = D // H                   # 64
    DC = D // P                   # 4
    KC = SK // P                  # key chunks
    NQT = SQ // P                 # query subtiles of 128
    QF = min(512, SQ)             # moving free size for q
    NQF = SQ // QF
    HPC = P // DK                 # heads per chunk (2)
    JB = min(4, NQT)              # qs-per-psum-bank in C accum
    NQB = NQT // JB               # C accum banks
    assert NQT % JB == 0
    Alu = mybir.AluOpType
    Act = mybir.ActivationFunctionType

    nc = bacc.Bacc("TRN2", target_bir_lowering=False, debug=False,
                   num_devices=num_devices)

    def din(name, shape, dt=FP32):
        return nc.dram_tensor(name, list(shape), dt, kind="ExternalInput").ap()

    qT = din("qT", (D, SQ), BF16)
    kT = din("kT", (D, SK), BF16)
    vT = din("vT", (D, SK), BF16)
    resid = din("resid", (SQ, D))
    w_dram = {n: din(n, (D, D), BF16) for n in ("wq", "wk", "wv", "wo")}
    bq_l = din("bq_l", (P, DC))
    bk_l = din("bk_l", (P, DC))
    bv_r = din("bv_r", (P, D))
    bo_r = din("bo_r", (P, D))
    g_r = din("g_r", (P, D))
    b_r = din("b_r", (P, D))
    out = nc.dram_tensor("out", [SQ, D], FP32, kind="ExternalOutput").ap()

    def f32r(ap):
        return ap  # bf16 operands: PE full rate, no fp32r rounding constraints

    with tile.TileContext(nc) as tc, ExitStack() as ctx:
        # ---------------- resident SBUF ----------------
        consts = ctx.enter_context(tc.tile_pool(name="consts", bufs=1))
        acts = ctx.enter_context(tc.tile_pool(name="acts", bufs=1))

        ident = consts.tile([P, P], FP32, tag="ident")
        make_identity(nc, ident[:])
        eps_sb = consts.tile([P, 1], FP32, tag="eps")
        nc.vector.memset(eps_sb[:], 1e-5)

        def load_const(tag, src, shape):
            t = consts.tile(list(shape), FP32, tag=tag)
            nc.sync.dma_start(t[:], src)
            return t

        bq_sb = load_const("bq", bq_l, (P, DC))
        bk_sb = load_const("bk", bk_l, (P, DC))
        bv_sb = load_const("bv", bv_r, (P, D))
        bo_sb = load_const("bo", bo_r, (P, D))
        g_sb = load_const("g", g_r, (P, D))
        b_sb = load_const("b", b_r, (P, D))
        wo_sb = consts.tile([P, DC, D], BF16, tag="wo")
        nc.sync.dma_start(wo_sb[:], w_dram["wo"].rearrange("(c p) o -> p c o", p=P))

        Qt = acts.tile([P, DC, SQ], BF16, tag="Qt")
        Kt = acts.tile([P, DC, SK], BF16, tag="Kt")
        V = acts.tile([P, KC, H, DK + 1], BF16, tag="V")
        C = acts.tile([P, NQT, D], FP32, tag="C")

        # ------- phase 1 chunks (woven into attention for overlap) -------
        # PSUM: st pool slots double as projection accumulators (tag sharing);
        # st bufs=2 (4 banks) + cps bufs=2 (4 banks) = all 8 banks.
        ctx2 = ctx.enter_context(ExitStack())
        p1_w = ctx2.enter_context(tc.tile_pool(name="p1_w", bufs=1))
        p1_in = ctx2.enter_context(tc.tile_pool(name="p1_in", bufs=2))
        st_ps = ctx2.enter_context(tc.tile_pool(name="st_ps", bufs=2, space="PSUM"))
        c_ps = ctx2.enter_context(tc.tile_pool(name="c_ps", bufs=2, space="PSUM"))
        pt_pool = ctx2.enter_context(tc.tile_pool(name="pt", bufs=3))
        lr_pool = ctx2.enter_context(tc.tile_pool(name="lr", bufs=2))

        wq_sb = p1_w.tile([P, DC, D], BF16, tag="wq")
        nc.sync.dma_start(wq_sb[:], w_dram["wq"].rearrange("(c p) o -> p c o", p=P))
        wk_sb = p1_w.tile([P, DC, D], BF16, tag="wk")
        nc.sync.dma_start(wk_sb[:], w_dram["wk"].rearrange("(c p) o -> p c o", p=P))
        wv_sb = p1_w.tile([P, DC, D], BF16, tag="wv")
        nc.sync.dma_start(wv_sb[:], w_dram["wv"].rearrange("(c p) o -> p c o", p=P))
        qT_sb = p1_w.tile([P, DC, SQ], BF16, tag="qTin")
        nc.sync.dma_start(qT_sb[:], qT.rearrange("(c p) q -> p c q", p=P))

        def proj_ps():
            return st_ps.tile([P, SQ], FP32, tag="st", name="projps")[:, 0:512]

        def q_chunk(dc_o):
            for qf in range(NQF):
                ps = proj_ps()[:, 0:QF]
                for dc_i in range(DC):
                    nc.tensor.matmul(
                        ps,
                        lhsT=f32r(wq_sb[:, dc_i, dc_o * P:(dc_o + 1) * P]),
                        rhs=f32r(qT_sb[:, dc_i, qf * QF:(qf + 1) * QF]),
                        start=(dc_i == 0), stop=(dc_i == DC - 1))
                nc.vector.tensor_scalar_add(
                    Qt[:, dc_o, qf * QF:(qf + 1) * QF], ps,
                    bq_sb[:, dc_o:dc_o + 1])

        SB = min(512, SK)
        NSB = SK // SB
        kin_tiles = {}

        def k_dma(sb):
            if sb in kin_tiles or sb >= NSB:
                return
            kin = p1_in.tile([P, DC, SB], BF16, tag="kin")
            nc.sync.dma_start(
                kin[:],
                kT.rearrange("(c p) s -> p c s", p=P)[:, :, sb * SB:(sb + 1) * SB])
            kin_tiles[sb] = kin

        def k_chunk(sb):
            k_dma(sb)
            kin = kin_tiles.pop(sb)
            for dc_o in range(DC):
                ps = proj_ps()[:, 0:SB]
                for dc_i in range(DC):
                    nc.tensor.matmul(
                        ps,
                        lhsT=f32r(wk_sb[:, dc_i, dc_o * P:(dc_o + 1) * P]),
                        rhs=f32r(kin[:, dc_i, :]),
                        start=(dc_i == 0), stop=(dc_i == DC - 1))
                nc.vector.tensor_scalar_add(
                    Kt[:, dc_o, sb * SB:(sb + 1) * SB], ps,
                    bk_sb[:, dc_o:dc_o + 1])
            k_dma(sb + 1)

        def v_chunk(sb):
            for sc in range(sb * (SB // P), (sb + 1) * (SB // P)):
                vin = p1_in.tile([P, DC, P], BF16, tag="vin")
                nc.sync.dma_start(
                    vin[:],
                    vT.rearrange("(c p) s -> p c s", p=P)[:, :, sc * P:(sc + 1) * P])
                ps = proj_ps()[:, 0:D]
                for dc_i in range(DC):
                    nc.tensor.matmul(
                        ps,
                        lhsT=f32r(vin[:, dc_i, :]),
                        rhs=f32r(wv_sb[:, dc_i, :]),
                        start=(dc_i == 0), stop=(dc_i == DC - 1))
                nc.vector.tensor_tensor(
                    V[:, sc, :, 0:DK],
                    ps.rearrange("p (h d) -> p h d", d=DK),
                    bv_sb[:].rearrange("p (h d) -> p h d", d=DK),
                    Alu.add)
                nc.vector.memset(V[:, sc, :, DK], 1.0)

        # ---------------- phase 2: attention (software-pipelined) ----------------
        # PE emission order per kc: St(kc) ... PV(kc-1), so the PE computes the
        # next St while ACT runs exp(kc) -- ACT stays the pacing engine.
        q_chunk(0)
        q_chunk(1)
        k_chunk(0)
        v_chunk(0)
        weave = {4 * i: i for i in range(1, NSB)}

        def emit_pv(h, kc, pt):
            for qs in range(NQT):
                jcol = (qs % JB) * (DK + 1)
                nc.tensor.matmul(
                    cps[:, qs // JB, jcol:jcol + DK + 1],
                    lhsT=pt[:, qs * P:(qs + 1) * P],
                    rhs=V[:, kc, h, :],
                    start=(kc == 0 and qs % JB == 0),
                    stop=(kc == KC - 1 and qs % JB == JB - 1))

        for h in range(H):
            dc_h = h // HPC
            off = (h % HPC) * DK
            cps = c_ps.tile([P, NQB, 512], FP32, tag="cps")
            pending = None
            for kc in range(KC):
                if h == 0 and kc in weave:
                    if pending is not None:
                        emit_pv(h, pending[0], pending[1])
                        pending = None
                    k_chunk(weave[kc])
                    v_chunk(weave[kc])
                if h == 1 and kc == 0:
                    q_chunk(2)
                    q_chunk(3)
                st = st_ps.tile([P, SQ], FP32, tag="st")
                for qf in range(NQF):
                    nc.tensor.matmul(
                        st[:, qf * QF:(qf + 1) * QF],
                        lhsT=f32r(Kt[off:off + DK, dc_h, kc * P:(kc + 1) * P]),
                        rhs=f32r(Qt[off:off + DK, dc_h, qf * QF:(qf + 1) * QF]),
                        start=True, stop=True)
                pt = pt_pool.tile([P, SQ], BF16, tag="pt")
                nc.scalar.activation(pt[:], st[:], Act.Exp, scale=0.125)
                if pending is not None:
                    emit_pv(h, pending[0], pending[1])
                pending = (kc, pt)
            emit_pv(h, pending[0], pending[1])
            # normalize + dump: C[:, :, h*DK:+DK] = C_ps[:, :, j, :DK] / L
            cview = cps[:, :, 0:JB * (DK + 1)].rearrange(
                "p b (j x) -> p b j x", x=DK + 1)
            lr = lr_pool.tile([P, NQB, JB, 1], FP32, tag="lr")
            nc.vector.reciprocal(lr[:], cview[:, :, :, DK:DK + 1])
            nc.vector.tensor_tensor(
                C[:, :, h * DK:(h + 1) * DK].rearrange(
                    "p (b j) d -> p b j d", j=JB),
                cview[:, :, :, 0:DK],
                lr[:].to_broadcast((P, NQB, JB, DK)),
                Alu.mult)

        ctx2.close()

        # ---------------- phases 3+4: transpose, out-proj, epilogue ----------------
        with tc.tile_pool(name="p3", bufs=1) as p3, \
             tc.tile_pool(name="tp_ps", bufs=2, space="PSUM") as tp_ps, \
             tc.tile_pool(name="o_ps", bufs=2, space="PSUM") as o_ps, \
             tc.tile_pool(name="ep", bufs=3) as ep:

            Ct = p3.tile([P, DC, SQ], BF16, tag="Ct")
            for dc in range(DC):
                for qs in range(NQT):
                    tp = tp_ps.tile([P, P], FP32, tag="tp")
                    nc.tensor.transpose(tp[:], C[:, qs, dc * P:(dc + 1) * P], ident[:])
                    nc.vector.tensor_copy(out=Ct[:, dc, qs * P:(qs + 1) * P], in_=tp[:])

            t0_all = p3.tile([P, NQT, D], FP32, tag="t0")
            mv_all = p3.tile([P, NQT, 2], FP32, tag="mv")
            rstd_all = p3.tile([P, NQT], FP32, tag="rstd")
            sdev = p3.tile([P, NQT], FP32, tag="sdev")

            for qs in range(NQT):
                ps = o_ps.tile([P, D], FP32, tag="ops")
                for dc in range(DC):
                    nc.tensor.matmul(
                        ps[:],
                        lhsT=f32r(Ct[:, dc, qs * P:(qs + 1) * P]),
                        rhs=f32r(wo_sb[:, dc, :]),
                        start=(dc == 0), stop=(dc == DC - 1))
                rs = ep.tile([P, D], FP32, tag="rs")
                nc.sync.dma_start(rs[:], resid[qs * P:(qs + 1) * P, :])
                t0 = t0_all[:, qs, :]
                nc.vector.tensor_tensor(t0, ps[:], bo_sb[:], Alu.add)
                nc.vector.tensor_tensor(t0, t0, rs[:], Alu.add)
                st6 = ep.tile([P, 6], FP32, tag="st6")
                nc.vector.bn_stats(st6[:], t0)
                nc.vector.bn_aggr(mv_all[:, qs, :], st6[:])

            nc.scalar.activation(sdev[:], mv_all[:, :, 1], Act.Sqrt, bias=eps_sb[:])
            nc.vector.reciprocal(rstd_all[:], sdev[:])

            for qs in range(NQT):
                t1 = ep.tile([P, D], FP32, tag="t1")
                nc.vector.tensor_scalar(
                    t1[:], t0_all[:, qs, :],
                    scalar1=mv_all[:, qs, 0:1], scalar2=rstd_all[:, qs:qs + 1],
                    op0=Alu.subtract, op1=Alu.mult)
                nc.vector.tensor_tensor(t1[:], t1[:], g_sb[:], Alu.mult)
                nc.vector.tensor_tensor(t1[:], t1[:], b_sb[:], Alu.add)
                nc.sync.dma_start(out[qs * P:(qs + 1) * P, :], t1[:])

    nc.compile()
    return nc


def make_in_map(query_slice, key_b, value_b, wq, bq, wk, bk, wv, bv, wo, bo,
                ln_g, ln_b):
    """Host-side shard prep for one core. query_slice [SQ, D]; key_b/value_b [SK, D]."""
    import numpy as np
    D = wq.shape[0]
    DC = D // P
    f = np.float32

    def rep(v):
        return np.ascontiguousarray(np.broadcast_to(v.astype(f), (P, D)))

    def plat(v):
        return np.ascontiguousarray(v.astype(f).reshape(DC, P).T)

    import ml_dtypes
    bf = ml_dtypes.bfloat16
    return {
        "qT": np.ascontiguousarray(query_slice.T.astype(f).astype(bf)),
        "kT": np.ascontiguousarray(key_b.T.astype(f).astype(bf)),
        "vT": np.ascontiguousarray(value_b.T.astype(f).astype(bf)),
        "resid": np.ascontiguousarray(query_slice.astype(f)),
        "wq": np.ascontiguousarray(wq.astype(f).astype(bf)),
        "wk": np.ascontiguousarray(wk.astype(f).astype(bf)),
        "wv": np.ascontiguousarray(wv.astype(f).astype(bf)),
        "wo": np.ascontiguousarray(wo.astype(f).astype(bf)),
        "bq_l": plat(bq), "bk_l": plat(bk),
        "bv_r": rep(bv), "bo_r": rep(bo), "g_r": rep(ln_g), "b_r": rep(ln_b),
    }


_NC_CACHE = {}


def _get_nc():
    if "nc" not in _NC_CACHE:
        _NC_CACHE["nc"] = build(SQ=1024, SK=2048, D=512, H=8, num_devices=8)
    return _NC_CACHE["nc"]


def kernel(query, key, value, wq, bq, wk, bk, wv, bv, wo, bo, ln_g, ln_b):
    from concourse.bass_utils import run_bass_kernel_spmd
    query = np.asarray(query, dtype=np.float32)
    key = np.asarray(key, dtype=np.float32)
    value = np.asarray(value, dtype=np.float32)
    B, SQ_FULL, D = query.shape
    SQH = SQ_FULL // 2
    nc = _get_nc()
    in_maps = []
    for c in range(8):
        b, qh = c // 2, c % 2
        in_maps.append(make_in_map(
            query[b, qh * SQH:(qh + 1) * SQH, :], key[b], value[b],
            np.asarray(wq), np.asarray(bq), np.asarray(wk), np.asarray(bk),
            np.asarray(wv), np.asarray(bv), np.asarray(wo), np.asarray(bo),
            np.asarray(ln_g), np.asarray(ln_b)))
    res = run_bass_kernel_spmd(nc, in_maps, core_ids=list(range(8)))
    out = np.empty((B, SQ_FULL, D), np.float32)
    for c, r in enumerate(res.results):
        out[c // 2, (c % 2) * SQH:((c % 2) + 1) * SQH, :] = r["out"]
    return out



# revision 5
# speedup vs baseline: 1.1327x; 1.1327x over previous
"""Trainium2 Bass kernel for nn_CrossModalAttention (B=4, Sq=Sk=2048, D=512, H=8).

Self-contained: builds an 8-core SPMD Bass/Tile program (one NeuronCore per
(batch, query-half) shard), compiles once per process, runs via
run_bass_kernel_spmd. See build() docstring for the on-chip algorithm.
"""
import sys
sys.path.insert(0, "/opt/trn_rl_repo")
import numpy as np

"""Builder for the CrossModalAttention Trainium2 kernel (one NeuronCore's shard).

Sharding: core c handles batch b=c//2, query-half qh=c%2 (SQ=1024 of 2048 queries).
Cores are fully independent (K/V projection duplicated across the core-pair of a
batch); no collectives.

Single software-pipelined (h, kc) stream paced by the ACT engine (exp is the
roofline: 8 heads x 16 key-chunks x [128,1024] activations ~ 132us). All other
work (Q/K/V projections, PV accumulation, per-head normalize, per-pair
transposes) is woven into PE/DVE/Pool idle slots at explicit deadlines:

  Qt [128, DC, SQ]  transposed Q-proj; head h in chunk h//2, rows (h%2)*64..+64
  Kt [128, DC, SK]  transposed K-proj
  V  [128, KC, H, DK+1] bf16; column DK is ones (row-sum trick)
  per (h, kc):
    St[k,q] = Kt_h(kc)^T @ Qt_h      PSUM [128, SQ]
    Pt      = exp(St/8)              ACT -> SBUF bf16 (pt ring, 32 deep)
    PV (lagged 1-2 heads): C_ps += Pt^T @ [V_h | 1]  per-head PSUM accum
  per head: C[:, :, h*64:+64] = C_ps * (1/L)   (DVE, bf16)
  per head-pair: PE-transpose C chunk -> Ct bf16
  tail per qs: out_ps = I^T@resid_plus + sum_dc Ct^T @ wo; LayerNorm via
  bn_stats (DVE) + Sqrt/Identity (ACT) + g-mult (DVE bf16) + b-add (Pool).

resid_plus = query + bo folded on host. Output [SQ, D] f32.
"""

from contextlib import ExitStack

import concourse.bass as bass
import concourse.mybir as mybir
import concourse.tile as tile
from concourse import bacc
from concourse.masks import make_identity

FP32 = mybir.dt.float32
BF16 = mybir.dt.bfloat16
P = 128


def build(SQ=1024, SK=2048, D=512, H=8, num_devices=8):
    DK = D // H                   # 64
    DC = D // P                   # 4
    KC = SK // P                  # 16 key chunks
    NQT = SQ // P                 # 8 query subtiles
    QF = 512                      # q free-size per St matmul
    NQF = SQ // QF                # 2
    HPC = P // DK                 # heads per 128-chunk (2)
    JB = 4                        # qs per psum-bank in C accum
    NQB = NQT // JB               # 2
    SB = 512                      # K/V projection block
    NSB = SK // SB                # 4
    Alu = mybir.AluOpType
    Act = mybir.ActivationFunctionType

    nc = bacc.Bacc("TRN2", target_bir_lowering=False, debug=False,
                   num_devices=num_devices)

    def din(name, shape, dt=FP32):
        return nc.dram_tensor(name, list(shape), dt, kind="ExternalInput").ap()

    qT = din("qT", (D, SQ), BF16)
    kT = din("kT", (D, SK), BF16)
    vT = din("vT", (D, SK), BF16)
    residb = din("residb", (SQ, D), BF16)      # query + bo, host-folded
    w_dram = {n: din(n, (D, D), BF16) for n in ("wq", "wk", "wv", "wo")}
    bq_l = din("bq_l", (P, DC))
    bk_l = din("bk_l", (P, DC))
    bv_r = din("bv_r", (P, D))
    g_b = din("g_b", (P, D), BF16)
    b_r = din("b_r", (P, D))
    out = nc.dram_tensor("out", [SQ, D], FP32, kind="ExternalOutput").ap()

    with tile.TileContext(nc) as tc, ExitStack() as ctx:
        # ---------------- resident SBUF ----------------
        consts = ctx.enter_context(tc.tile_pool(name="consts", bufs=1))
        acts = ctx.enter_context(tc.tile_pool(name="acts", bufs=1))
        p1_in = ctx.enter_context(tc.tile_pool(name="p1_in", bufs=4))
        vin_pool = ctx.enter_context(tc.tile_pool(name="vin", bufs=2))
        pt_pool = ctx.enter_context(tc.tile_pool(name="pt", bufs=32))
        lr_pool = ctx.enter_context(tc.tile_pool(name="lr", bufs=2))
        ep = ctx.enter_context(tc.tile_pool(name="ep", bufs=3))
        st_ps = ctx.enter_context(tc.tile_pool(name="st_ps", bufs=2, space="PSUM"))
        c_ps = ctx.enter_context(tc.tile_pool(name="c_ps", bufs=2, space="PSUM"))

        identB = consts.tile([P, P], BF16, tag="identB")
        make_identity(nc, identB[:])
        identF = consts.tile([P, P], FP32, tag="identF")
        make_identity(nc, identF[:])
        eps_sb = consts.tile([P, 1], FP32, tag="eps")
        nc.vector.memset(eps_sb[:], 1e-5)

        # -------- DMA prologue (order = priority on the sync queue) --------
        wq_sb = consts.tile([P, DC, D], BF16, tag="wq")
        wk_sb = consts.tile([P, DC, D], BF16, tag="wk")
        wv_sb = consts.tile([P, DC, D], BF16, tag="wv")
        wo_sb = consts.tile([P, DC, D], BF16, tag="wo")
        qT_sb = consts.tile([P, DC, SQ], BF16, tag="qTin")

        nc.sync.dma_start(wk_sb[:], w_dram["wk"].rearrange("(c p) o -> p c o", p=P))
        kin = []
        for sb in range(NSB):
            t = p1_in.tile([P, DC, SB], BF16, tag="kin")
            nc.sync.dma_start(
                t[:], kT.rearrange("(c p) s -> p c s", p=P)[:, :, sb * SB:(sb + 1) * SB])
            kin.append(t)
            if sb == 1:
                nc.sync.dma_start(
                    wq_sb[:], w_dram["wq"].rearrange("(c p) o -> p c o", p=P))
                nc.sync.dma_start(qT_sb[:], qT.rearrange("(c p) q -> p c q", p=P))
        nc.sync.dma_start(wv_sb[:], w_dram["wv"].rearrange("(c p) o -> p c o", p=P))
        bq_sb = consts.tile([P, DC], FP32, tag="bq")
        nc.sync.dma_start(bq_sb[:], bq_l)
        bk_sb = consts.tile([P, DC], FP32, tag="bk")
        nc.sync.dma_start(bk_sb[:], bk_l)
        bv_sb = consts.tile([P, D], FP32, tag="bv")
        nc.sync.dma_start(bv_sb[:], bv_r)

        vin = [None] * NSB

        def v_dma(j):
            if j >= NSB or vin[j] is not None:
                return
            vin[j] = vin_pool.tile([P, DC, SB], BF16, tag="vin", name="vin")
            nc.sync.dma_start(
                vin[j][:],
                vT.rearrange("(c p) s -> p c s", p=P)[:, :, j * SB:(j + 1) * SB])

        v_dma(0)
        nc.sync.dma_start(wo_sb[:], w_dram["wo"].rearrange("(c p) o -> p c o", p=P))
        g_sb = consts.tile([P, D], BF16, tag="g")
        nc.sync.dma_start(g_sb[:], g_b)
        b_sb = consts.tile([P, D], FP32, tag="b")
        nc.sync.dma_start(b_sb[:], b_r)

        # -------- persistent activation tensors --------
        Qt = acts.tile([P, DC, SQ], BF16, tag="Qt")
        Kt = acts.tile([P, DC, SK], BF16, tag="Kt")
        V = acts.tile([P, KC, H, DK + 1], BF16, tag="V")
        C = acts.tile([P, NQT, D], FP32, tag="C")
        Ct = acts.tile([P, DC, SQ], BF16, tag="Ct")

        def stile():
            return st_ps.tile([P, SQ], FP32, tag="st", name="st")

        # -------- woven work-item helpers --------
        def kproj(sb, dc_o):
            ps = stile()[:, 0:SB]
            for dc_i in range(DC):
                nc.tensor.matmul(
                    ps, lhsT=wk_sb[:, dc_i, dc_o * P:(dc_o + 1) * P],
                    rhs=kin[sb][:, dc_i, :],
                    start=(dc_i == 0), stop=(dc_i == DC - 1))
            nc.vector.tensor_scalar_add(
                Kt[:, dc_o, sb * SB:(sb + 1) * SB], ps, bk_sb[:, dc_o:dc_o + 1])

        def qproj(dc_o, qf):
            ps = stile()[:, 0:QF]
            for dc_i in range(DC):
                nc.tensor.matmul(
                    ps, lhsT=wq_sb[:, dc_i, dc_o * P:(dc_o + 1) * P],
                    rhs=qT_sb[:, dc_i, qf * QF:(qf + 1) * QF],
                    start=(dc_i == 0), stop=(dc_i == DC - 1))
            nc.vector.tensor_scalar_add(
                Qt[:, dc_o, qf * QF:(qf + 1) * QF], ps, bq_sb[:, dc_o:dc_o + 1])

        def vproj(sc):
            j = sc * P // SB
            v_dma(j + 1)
            ps = stile()[:, 0:D]
            for dc_i in range(DC):
                nc.tensor.matmul(
                    ps, lhsT=vin[j][:, dc_i, sc * P - j * SB:(sc + 1) * P - j * SB],
                    rhs=wv_sb[:, dc_i, :],
                    start=(dc_i == 0), stop=(dc_i == DC - 1))
            nc.vector.tensor_tensor(
                V[:, sc, :, 0:DK],
                ps.rearrange("p (h d) -> p h d", d=DK),
                bv_sb[:].rearrange("p (h d) -> p h d", d=DK), Alu.add)
            nc.vector.memset(V[:, sc, :, DK], 1.0)

        pts = {}
        cps_of = {}

        def emit_pv(h, kc):
            if kc == 0:
                cps_of[h] = c_ps.tile([P, NQB, 512], FP32, tag="cps", name="cps")
            cps = cps_of[h]
            pt = pts.pop((h, kc))
            for qs in range(NQT):
                jcol = (qs % JB) * (DK + 1)
                nc.tensor.matmul(
                    cps[:, qs // JB, jcol:jcol + DK + 1],
                    lhsT=pt[:, qs * P:(qs + 1) * P],
                    rhs=V[:, kc, h, :],
                    start=(kc == 0 and qs % JB == 0),
                    stop=(kc == KC - 1 and qs % JB == JB - 1))

        def norm(h):
            cps = cps_of.pop(h)
            cview = cps[:, :, 0:JB * (DK + 1)].rearrange(
                "p b (j x) -> p b j x", x=DK + 1)
            lr = lr_pool.tile([P, NQB, JB, 1], FP32, tag="lr")
            nc.vector.reciprocal(lr[:], cview[:, :, :, DK:DK + 1])
            nc.vector.tensor_tensor(
                C[:, :, h * DK:(h + 1) * DK].rearrange(
                    "p (b j) d -> p b j d", j=JB),
                cview[:, :, :, 0:DK],
                lr[:].to_broadcast((P, NQB, JB, DK)), Alu.mult)

        def tpose(dc, qs):
            tp = stile()[:, 0:P]
            nc.tensor.transpose(tp, C[:, qs, dc * P:(dc + 1) * P], identF[:])
            nc.vector.tensor_copy(out=Ct[:, dc, qs * P:(qs + 1) * P], in_=tp)

        # weave tables: (h, kc) -> work items emitted after St/exp of that step
        post = {}

        def add(h, kc, fn, *a):
            post.setdefault((h, kc), []).append((fn, a))

        for i, sb in enumerate(range(1, NSB)):
            add(0, i, kproj, sb, 0)          # dc0 for sb1-3: steps 0,1,2
            add(0, 4 + 2 * i, kproj, sb, 1)  # dc1: steps 4,6,8
            add(1, 1 + 2 * i, kproj, sb, 2)  # dc2: h1 steps 1,3,5
            add(2, 1 + 2 * i, kproj, sb, 3)  # dc3: h2 steps 1,3,5
        add(0, 10, qproj, 1, 0)
        add(0, 12, qproj, 1, 1)
        add(1, 7, qproj, 2, 0)
        add(1, 9, qproj, 2, 1)
        add(2, 7, qproj, 3, 0)
        add(2, 9, qproj, 3, 1)
        for j in range(8):
            add(1, 2 * j, vproj, j)          # sc 0-7 on h1 even steps
            add(2, 2 * j, vproj, 8 + j)      # sc 8-15 on h2 even steps
        for i, (h, pair) in enumerate([(4, 0), (6, 1), (7, 2)]):
            for qs in range(NQT):
                add(h, 4 + qs, tpose, pair, qs)
        for h in range(7):
            add(h + 1 + (h <= 3), 15, norm, h)   # norm(h) after PV(h,15)

        # PV emission: pre-St at the consumer step. lag-2 heads 0-3, lag-1 4-6,
        # lag-4kc head 7 (pt ring is 32 deep = 2 heads).
        pv_at = {}
        for h in range(4):
            for kc in range(KC):
                pv_at.setdefault((h + 2, kc), []).append((h, kc))
        for h in (4, 5, 6):
            for kc in range(KC):
                pv_at.setdefault((h + 1, kc), []).append((h, kc))
        for kc in range(KC - 4):
            pv_at.setdefault((7, kc + 4), []).append((7, kc))

        # -------- prologue compute: K-proj sb0 + Q-proj dc0 --------
        for dc_o in range(DC):
            kproj(0, dc_o)
        qproj(0, 0)
        qproj(0, 1)

        # -------- main (h, kc) stream --------
        for h in range(H):
            dc_h = h // HPC
            off = (h % HPC) * DK
            for kc in range(KC):
                for (hp, kcp) in pv_at.get((h, kc), ()):
                    emit_pv(hp, kcp)
                st = stile()
                for qf in range(NQF):
                    nc.tensor.matmul(
                        st[:, qf * QF:(qf + 1) * QF],
                        lhsT=Kt[off:off + DK, dc_h, kc * P:(kc + 1) * P],
                        rhs=Qt[off:off + DK, dc_h, qf * QF:(qf + 1) * QF],
                        start=True, stop=True)
                pt = pt_pool.tile([P, SQ], BF16, tag="pt")
                nc.scalar.activation(pt[:], st[:], Act.Exp, scale=0.125)
                pts[(h, kc)] = pt
                for fn, a in post.get((h, kc), ()):
                    fn(*a)

        # -------- tail: PV(7) leftovers, norm(7), tp pair 3, out-proj + LN ----
        for kc in range(KC - 4, KC):
            emit_pv(7, kc)
        norm(7)
        for qs in range(NQT):
            tpose(3, qs)

        res_t = []
        for qs in range(NQT):
            rs = ep.tile([P, D], BF16, tag="rs")
            nc.sync.dma_start(rs[:], residb[qs * P:(qs + 1) * P, :])
            res_t.append(rs)

        for qs in range(NQT):
            ops = stile()[:, 0:D]
            nc.tensor.matmul(ops, lhsT=identB[:], rhs=res_t[qs][:],
                             start=True, stop=False)
            for dc in range(DC):
                nc.tensor.matmul(
                    ops, lhsT=Ct[:, dc, qs * P:(qs + 1) * P],
                    rhs=wo_sb[:, dc, :], start=False, stop=(dc == DC - 1))
            st6 = ep.tile([P, 6], FP32, tag="st6")
            nc.vector.bn_stats(st6[:], ops)
            mv = ep.tile([P, 2], FP32, tag="mv")
            nc.vector.bn_aggr(mv[:], st6[:])
            sdev = ep.tile([P, 1], FP32, tag="sdev")
            nc.scalar.activation(sdev[:], mv[:, 1:2], Act.Sqrt, bias=eps_sb[:])
            rstd = ep.tile([P, 1], FP32, tag="rstd")
            nc.vector.reciprocal(rstd[:], sdev[:])
            nmr = ep.tile([P, 1], FP32, tag="nmr")
            nc.vector.tensor_tensor(nmr[:], mv[:, 0:1], rstd[:], Alu.mult)
            nc.vector.tensor_scalar_mul(nmr[:], nmr[:], -1.0)
            tb = ep.tile([P, D], BF16, tag="tb")
            nc.scalar.activation(tb[:], ops, Act.Identity,
                                 bias=nmr[:], scale=rstd[:])
            t2 = ep.tile([P, D], BF16, tag="t2")
            nc.vector.tensor_tensor(t2[:], tb[:], g_sb[:], Alu.mult)
            tout = ep.tile([P, D], FP32, tag="tout")
            nc.gpsimd.tensor_tensor(tout[:], t2[:], b_sb[:], Alu.add)
            nc.sync.dma_start(out[qs * P:(qs + 1) * P, :], tout[:])

    nc.compile()
    return nc


def make_in_map(query_slice, key_b, value_b, wq, bq, wk, bk, wv, bv, wo, bo,
                ln_g, ln_b):
    """Host-side shard prep for one core. query_slice [SQ, D]; key_b/value_b [SK, D]."""
    import numpy as np
    D = wq.shape[0]
    DC = D // P
    f = np.float32

    def rep(v, dt=np.float32):
        return np.ascontiguousarray(np.broadcast_to(v.astype(f), (P, D)).astype(dt))

    def plat(v):
        return np.ascontiguousarray(v.astype(f).reshape(DC, P).T)

    import ml_dtypes
    bf = ml_dtypes.bfloat16
    return {
        "qT": np.ascontiguousarray(query_slice.T.astype(f).astype(bf)),
        "kT": np.ascontiguousarray(key_b.T.astype(f).astype(bf)),
        "vT": np.ascontiguousarray(value_b.T.astype(f).astype(bf)),
        "residb": np.ascontiguousarray(
            (query_slice.astype(f) + bo.astype(f)).astype(bf)),
        "wq": np.ascontiguousarray(wq.astype(f).astype(bf)),
        "wk": np.ascontiguousarray(wk.astype(f).astype(bf)),
        "wv": np.ascontiguousarray(wv.astype(f).astype(bf)),
        "wo": np.ascontiguousarray(wo.astype(f).astype(bf)),
        "bq_l": plat(bq), "bk_l": plat(bk),
        "bv_r": rep(bv), "g_b": rep(ln_g, bf), "b_r": rep(ln_b),
    }


_NC_CACHE = {}


def _get_nc():
    if "nc" not in _NC_CACHE:
        _NC_CACHE["nc"] = build(SQ=1024, SK=2048, D=512, H=8, num_devices=8)
    return _NC_CACHE["nc"]


def kernel(query, key, value, wq, bq, wk, bk, wv, bv, wo, bo, ln_g, ln_b):
    from concourse.bass_utils import run_bass_kernel_spmd
    query = np.asarray(query, dtype=np.float32)
    key = np.asarray(key, dtype=np.float32)
    value = np.asarray(value, dtype=np.float32)
    B, SQ_FULL, D = query.shape
    SQH = SQ_FULL // 2
    nc = _get_nc()
    in_maps = []
    for c in range(8):
        b, qh = c // 2, c % 2
        in_maps.append(make_in_map(
            query[b, qh * SQH:(qh + 1) * SQH, :], key[b], value[b],
            np.asarray(wq), np.asarray(bq), np.asarray(wk), np.asarray(bk),
            np.asarray(wv), np.asarray(bv), np.asarray(wo), np.asarray(bo),
            np.asarray(ln_g), np.asarray(ln_b)))
    res = run_bass_kernel_spmd(nc, in_maps, core_ids=list(range(8)))
    out = np.empty((B, SQ_FULL, D), np.float32)
    for c, r in enumerate(res.results):
        out[c // 2, (c % 2) * SQH:((c % 2) + 1) * SQH, :] = r["out"]
    return out
